# revision 19
# baseline (speedup 1.0000x reference)
"""Trainium2 Bass kernel for nn_MetricSelfAttention (v2, bf16).

Math: the reference's softmax is dead code, so
    nudged = (p1 @ M @ p2^T) @ p1
reassociates to
    nudged = p1 @ (M @ (p2^T @ p1))        (per-head 64x64 Gram matrix G)
collapsing the O(W^2) attention entirely.  The kernel is memory-bound.

Sharding: 8 cores = 2 batches x 4 head-pairs.  Core (b, hg) computes heads
{2hg, 2hg+1} of batch b and writes the partial mixer product
    out_partial = nudged[:, 128hg:+128] @ W_mixer[:, 128hg:+128].T
as bf16; the host sums the 4 partials per batch in fp32 and adds b_mixer.

All bulk tensors are bf16 (host-cast): halves HBM traffic vs fp32 and runs
the PE at 1 cycle/row irrespective of moving-dim size.  Per core the HBM
traffic is x1^T (2 MiB) + x2 (2 MiB) + out (2 MiB) + params (~0.3 MiB).

LayerNorm is computed on device and folded so no normalized tensor is ever
materialized:
  - x1 stats come from x1^T via PE ones-matmuls (mean and mean-square rows);
    the mean enters the projection as a rank-1 matmul (colsum (x) -mu), and
    1/std multiplies p1^T columns via one partition-broadcast row per quad
    (so the final output needs no row scaling at all: D1 rides inside p1T).
  - x2 is never normalized: with x2n = rstd2*(x2 - mu2) rowwise,
      F := x2n^T @ p1  ==  x2^T @ (rstd2*p1) - 1 (x) ((mu2*rstd2)^T @ p1)
    so F comes from raw x2 plus a rank-1 correction (v-row).
  - gamma folds into the projection on the host; nonzero beta enters as
    rank-1 bias matmuls (compiled only when beta != 0).

DMA: big transfers ride the two HWDGE queues (sync: x1t + stores,
scalar: x2 + stores), 512 KiB apiece; params go on gpsimd SWDGE.
"""

from contextlib import ExitStack

import numpy as np
import ml_dtypes

import concourse.bacc as bacc
import concourse.bass as bass
import concourse.tile as tile
from concourse import mybir
from concourse.bass_utils import run_bass_kernel_spmd
from concourse.masks import make_identity

B, W, C, N, K = 2, 2048, 512, 8, 64
NCORES = 8
HPC = 2          # heads per core
K2 = HPC * K     # 128 channels per core
EPS = 1e-5
FP32 = mybir.dt.float32
BF16 = mybir.dt.bfloat16
AF = mybir.ActivationFunctionType
OP = mybir.AluOpType
BF = ml_dtypes.bfloat16

NT = W // 128    # 16 w-tiles
NQ = W // 512    # 4 w-quads
NJ = C // 128    # 4 c-chunks


def _body(ctx: ExitStack, tc: tile.TileContext, x1td, x2d, pjd, colsumd, mmd,
          wmixTd, outd, pbrd, pbcd):
    nc = tc.nc
    with_pbias = pbrd is not None

    persist = ctx.enter_context(tc.tile_pool(name="persist", bufs=1))
    sqpool = ctx.enter_context(tc.tile_pool(name="sq", bufs=2))
    rowpool = ctx.enter_context(tc.tile_pool(name="rows", bufs=2))
    bcpool = ctx.enter_context(tc.tile_pool(name="bc", bufs=2))
    spool = ctx.enter_context(tc.tile_pool(name="stats", bufs=8))
    outpool = ctx.enter_context(tc.tile_pool(name="outstage", bufs=2))
    ps_st = ctx.enter_context(tc.tile_pool(name="ps_st", bufs=2, space="PSUM"))
    ps_mm = ctx.enter_context(tc.tile_pool(name="ps_mm", bufs=2, space="PSUM"))
    ps_tp = ctx.enter_context(tc.tile_pool(name="ps_tp", bufs=1, space="PSUM"))
    ps_f = ctx.enter_context(tc.tile_pool(name="ps_f", bufs=1, space="PSUM"))
    ps_mo = ctx.enter_context(tc.tile_pool(name="ps_mo", bufs=2, space="PSUM"))

    # ---- params: pj/colsum right behind the first x1t chunks on sync; the
    # rest trail x2 on scalar (needed only in the tail) -----------------------
    pj_s = persist.tile([128, NJ, K2], BF16)
    colsum_s = persist.tile([1, K2], BF16)
    wmixT_s = persist.tile([K2, C], BF16)
    mm_s = persist.tile([K, HPC, K], BF16)
    pbr_s = pbc_s = None
    if with_pbias:
        pbr_s = persist.tile([1, K2], BF16)
        nc.sync.dma_start(out=pbr_s, in_=pbrd)
        pbc_s = persist.tile([K2, 1], FP32)
        nc.sync.dma_start(out=pbc_s, in_=pbcd)

    # ---- constants ---------------------------------------------------------
    oneC = persist.tile([128, 1], BF16)
    nc.vector.memset(oneC, 1.0 / C)
    eps_s = persist.tile([128, 1], FP32)
    nc.vector.memset(eps_s, EPS)
    eps_row = persist.tile([1, 1], FP32)
    nc.vector.memset(eps_row, EPS)
    one11 = persist.tile([1, 1], FP32)
    nc.vector.memset(one11, 1.0)
    ident = persist.tile([128, 128], BF16)
    make_identity(nc, ident)

    # ---- persistent activations --------------------------------------------
    x1t_s = persist.tile([128, NJ, W], BF16)    # x1^T
    x2r_s = persist.tile([128, NT, C], BF16)    # raw x2
    p1T_s = persist.tile([K2, W], BF16)         # (D1 p1u)^T
    p1s_s = persist.tile([128, NT, K2], BF16)   # rstd1*rstd2 * p1u
    rstd2_s = persist.tile([128, NT], FP32)
    rstd1n_s = persist.tile([128, NT], FP32)    # rstd1, natural layout
    sn_s = persist.tile([128, NT], FP32)        # rstd1*rstd2, natural
    mu2_s = persist.tile([128, NT], BF16)       # mu2 (bf16 lhsT for v)
    ft_s = persist.tile([K2, C], BF16)          # F^T
    f_s = persist.tile([128, NJ, K2], BF16)     # F natural
    g_s = persist.tile([K, HPC * K], BF16)      # Gram (2 heads side by side)
    hbd_s = persist.tile([K2, K2], BF16)        # block-diag H
    nudgT_s = persist.tile([K2, W], BF16)       # nudged^T
    vrow_s = persist.tile([1, K2], BF16)
    neg_ones = persist.tile([1, 512], BF16)
    nc.vector.memset(neg_ones, -1.0)
    if with_pbias:
        s1_s = persist.tile([1, K2], BF16)
        std2_s = persist.tile([128, NT], BF16)

    nc.vector.memset(hbd_s, 0.0)

    # ========================================================================
    # Front end, pipelined per w-quad
    # ========================================================================
    ftp = ps_f.tile([K2, C], FP32, tag="f")
    for q in range(NQ):
        qs = slice(q * 512, (q + 1) * 512)

        if q == 0:
            for jj in range(NJ):
                nc.sync.dma_start(
                    out=x1t_s[:, jj:jj + 1, qs],
                    in_=x1td[128 * jj:128 * (jj + 1), qs].rearrange(
                        "(j p) w -> p j w", p=128))
            nc.sync.dma_start(out=pj_s,
                              in_=pjd.rearrange("(j p) k -> p j k", p=128))
            nc.sync.dma_start(out=colsum_s, in_=colsumd)
        else:
            nc.sync.dma_start(
                out=x1t_s[:, :, qs],
                in_=x1td[:, qs].rearrange("(j p) w -> p j w", p=128))
        nc.scalar.dma_start(
            out=x2r_s[:, 4 * q:4 * (q + 1), :],
            in_=x2d[qs, :].rearrange("(t p) c -> p t c", p=128))
        if q == NQ - 1:
            nc.scalar.dma_start(out=mm_s, in_=mmd)
            nc.scalar.dma_start(out=wmixT_s, in_=wmixTd)

        # x2 row stats: one bn_stats per quad, batched tail math
        st6 = spool.tile([128, 4, 6], FP32, tag="bst")
        mvq = spool.tile([128, 4, 2], FP32, tag="mv")
        for t in range(4):
            nc.vector.bn_stats(st6[:, t, :], x2r_s[:, 4 * q + t, :])
            nc.vector.bn_aggr(mvq[:, t, :], st6[:, t, :])
        stdq = spool.tile([128, 4], FP32, tag="std")
        nc.scalar.activation(stdq, mvq[:, :, 1], AF.Sqrt, bias=eps_s, scale=1.0)
        nc.vector.reciprocal(rstd2_s[:, 4 * q:4 * (q + 1)], stdq)
        nc.vector.tensor_copy(out=mu2_s[:, 4 * q:4 * (q + 1)], in_=mvq[:, :, 0])
        if with_pbias:
            nc.vector.tensor_copy(out=std2_s[:, 4 * q:4 * (q + 1)], in_=stdq)

        # x1 stats rows: mu = 1/C ones^T x1t ; ssq = 1/C ones^T (x1t^2)
        st_ps = ps_st.tile([33, 512], FP32, tag="st")
        mu_ps = st_ps[0:1, :]
        ssq_ps = st_ps[32:33, :]
        sq = sqpool.tile([128, NJ, 512], BF16, tag="sq")
        nc.vector.tensor_mul(sq[:, 0:2, :], x1t_s[:, 0:2, qs], x1t_s[:, 0:2, qs])
        nc.scalar.activation(sq[:, 2:4, :], x1t_s[:, 2:4, qs], AF.Square)

        sqsum = sqpool.tile([128, 2, 512], BF16, tag="sqsum")
        nc.vector.tensor_add(sqsum[:, 0, :], sq[:, 0, :], sq[:, 1, :])
        nc.vector.tensor_add(sqsum[:, 1, :], sq[:, 2, :], sq[:, 3, :])

        pt = ps_mm.tile([128, 512], FP32, tag="mm")
        for j in range(NJ):
            nc.tensor.matmul(mu_ps, lhsT=oneC, rhs=x1t_s[:, j, qs],
                             start=(j == 0), stop=(j == NJ - 1))
        for j in range(NJ):
            nc.tensor.matmul(pt, lhsT=pj_s[:, j, :], rhs=x1t_s[:, j, qs],
                             start=(j == 0), stop=False)
        for j in range(2):
            nc.tensor.matmul(ssq_ps, lhsT=oneC, rhs=sqsum[:, j, :],
                             start=(j == 0), stop=(j == 1))

        nmu = rowpool.tile([1, 512], BF16, tag="nmu")
        nc.vector.tensor_scalar_mul(nmu, mu_ps, -1.0)
        musq = rowpool.tile([1, 512], FP32, tag="musq")
        nc.scalar.activation(musq, nmu, AF.Square)
        varrow = rowpool.tile([1, 512], FP32, tag="var")
        nc.vector.tensor_sub(varrow, ssq_ps, musq)

        # projection tail: rank-1 centering
        nc.tensor.matmul(pt, lhsT=colsum_s, rhs=nmu, start=False, stop=True)

        if with_pbias:
            # row-domain rstd1: p1^T scaled (+ pbias) right here
            stdrow = rowpool.tile([1, 512], FP32, tag="stdr")
            nc.scalar.activation(stdrow, varrow, AF.Sqrt, bias=eps_row,
                                 scale=1.0)
            rstd1row = rowpool.tile([1, 512], FP32, tag="rs1")
            nc.vector.reciprocal_approx_fast(rstd1row, stdrow)
            bc1 = bcpool.tile([128, 512], FP32, tag="bc1")
            nc.gpsimd.partition_broadcast(bc1, rstd1row)
            tmp = bcpool.tile([128, 512], FP32, tag="ptmp")
            nc.vector.tensor_mul(tmp, pt, bc1)
            nc.vector.tensor_scalar_add(p1T_s[:, qs], tmp, pbc_s)
        else:
            # departition varrow -> [128, 4] via tiny PE outer products;
            # rstd1 finishes in natural layout and rides the out-copies
            vntile = ps_mo.tile([128, 512], FP32, tag="mo")
            vnat = vntile[:, 0:4]
            for t in range(4):
                nc.tensor.matmul(vnat[:, t:t + 1],
                                 lhsT=varrow[0:1, t * 128:(t + 1) * 128],
                                 rhs=one11)
            stdn = spool.tile([128, 4], FP32, tag="stdn")
            nc.scalar.activation(stdn, vnat, AF.Sqrt, bias=eps_s, scale=1.0)
            nc.vector.reciprocal(rstd1n_s[:, 4 * q:4 * (q + 1)], stdn)
            nc.vector.tensor_mul(sn_s[:, 4 * q:4 * (q + 1)],
                                 rstd1n_s[:, 4 * q:4 * (q + 1)],
                                 rstd2_s[:, 4 * q:4 * (q + 1)])
            nc.vector.tensor_copy(out=p1T_s[:, qs], in_=pt)

        # p1 natural (PE transpose), rstd2-scaled on the PSUM->SBUF copy
        tp = ps_tp.tile([128, 512], BF16, tag="tp")
        for t in range(4):
            w_t = 4 * q + t
            nc.tensor.transpose(tp[:, t * 128:(t + 1) * 128],
                                p1T_s[:, w_t * 128:(w_t + 1) * 128], ident)
        p1s_scale = rstd2_s if with_pbias else sn_s
        for t in range(4):
            tt = 4 * q + t
            nc.scalar.activation(p1s_s[:, tt, :], tp[:, t * 128:(t + 1) * 128],
                                 AF.Copy, bias=0.0,
                                 scale=p1s_scale[:, tt:tt + 1])

        # F^T accumulation for this quad's tiles
        for t in range(4):
            tt = 4 * q + t
            nc.tensor.matmul(ftp, lhsT=p1s_s[:, tt, :], rhs=x2r_s[:, tt, :],
                             start=(tt == 0), stop=False)

    # ========================================================================
    # Gram tail
    # ========================================================================
    # v = mu2^T @ p1s ; F^T -= v (x) 1
    vtile = ps_st.tile([33, 512], FP32, tag="st")
    vps = vtile[0:1, :K2]
    for t in range(NT):
        nc.tensor.matmul(vps, lhsT=mu2_s[:, t:t + 1], rhs=p1s_s[:, t, :],
                         start=(t == 0), stop=(t == NT - 1))
    nc.scalar.copy(out=vrow_s, in_=vps)
    nc.tensor.matmul(ftp, lhsT=vrow_s, rhs=neg_ones, start=False, stop=True)
    nc.scalar.copy(out=ft_s, in_=ftp)

    # F natural
    ftp2 = ps_tp.tile([128, 512], BF16, tag="tp")
    for j in range(NJ):
        nc.tensor.transpose(ftp2[:, j * 128:(j + 1) * 128],
                            ft_s[:, j * 128:(j + 1) * 128], ident)
    nc.scalar.copy(out=f_s, in_=ftp2)

    if with_pbias:
        sptile = ps_st.tile([33, 512], FP32, tag="st")
        sp = sptile[0:1, :K2]
        for t in range(NT):
            nc.tensor.matmul(sp, lhsT=std2_s[:, t:t + 1], rhs=p1s_s[:, t, :],
                             start=(t == 0), stop=(t == NT - 1))
        nc.scalar.copy(out=s1_s, in_=sp)

    # G_h = P_h^T @ F_h (+ pb_h (x) s1_h)
    gp = ps_mm.tile([128, 512], FP32, tag="mm")
    gv = gp[:K, :HPC * K]
    for h in range(HPC):
        hs = slice(h * K, (h + 1) * K)
        for j in range(NJ):
            nc.tensor.matmul(gv[:, hs], lhsT=pj_s[:, j, hs], rhs=f_s[:, j, hs],
                             start=(j == 0),
                             stop=(j == NJ - 1) and not with_pbias)
        if with_pbias:
            nc.tensor.matmul(gv[:, hs], lhsT=pbr_s[:, hs], rhs=s1_s[:, hs],
                             start=False, stop=True)
    nc.scalar.copy(out=g_s, in_=gv)

    # H_h = M_h @ G_h; assemble block-diagonal (hbd zeroed at start)
    hp = ps_mm.tile([128, 512], FP32, tag="mm")
    hv = hp[:K, :HPC * K]
    for h in range(HPC):
        hs = slice(h * K, (h + 1) * K)
        nc.tensor.matmul(hv[:, hs], lhsT=mm_s[:, h, :], rhs=g_s[:, hs])
    for h in range(HPC):
        hs = slice(h * K, (h + 1) * K)
        nc.vector.tensor_copy(out=hbd_s[hs, hs], in_=hv[:K, hs])

    # nudged^T = H_bd^T @ p1^T;  outT_j = wmixT_j^T @ nudged^T
    for q in range(NQ):
        qs = slice(q * 512, (q + 1) * 512)
        npp = ps_mm.tile([128, 512], FP32, tag="mm")
        nc.tensor.matmul(npp, lhsT=hbd_s, rhs=p1T_s[:, qs])
        nc.scalar.copy(out=nudgT_s[:, qs], in_=npp)

    for q in range(NQ):
        qs = slice(q * 512, (q + 1) * 512)
        stage = outpool.tile([128, 4, C], BF16, tag="ostage")
        for t in range(4):
            w_t = q * 4 + t
            mo = ps_mo.tile([128, 512], FP32, tag="mo")
            nc.tensor.matmul(mo, lhsT=nudgT_s[:, w_t * 128:(w_t + 1) * 128],
                             rhs=wmixT_s)
            if with_pbias:
                if t % 2 == 0:
                    nc.vector.tensor_copy(out=stage[:, t, :], in_=mo)
                else:
                    nc.scalar.copy(out=stage[:, t, :], in_=mo)
            elif t % 2 == 0:
                nc.vector.tensor_scalar_mul(stage[:, t, :], mo,
                                            rstd1n_s[:, w_t:w_t + 1])
            else:
                nc.scalar.activation(stage[:, t, :], mo, AF.Copy, bias=0.0,
                                     scale=rstd1n_s[:, w_t:w_t + 1])
        eng = nc.sync if q % 2 == 0 else nc.scalar
        eng.dma_start(
            out=outd[qs, :].rearrange("(t p) c -> p t c", p=128),
            in_=stage)


_PROGRAM_CACHE = {}


def _get_program(with_pbias: bool):
    key = ("v2", with_pbias)
    if key in _PROGRAM_CACHE:
        return _PROGRAM_CACHE[key]
    nc = bacc.Bacc("TRN2", debug=False, num_devices=NCORES)
    x1td = nc.dram_tensor("x1t", [C, W], BF16, kind="ExternalInput").ap()
    x2d = nc.dram_tensor("x2", [W, C], BF16, kind="ExternalInput").ap()
    pjd = nc.dram_tensor("pj", [C, K2], BF16, kind="ExternalInput").ap()
    colsumd = nc.dram_tensor("colsum", [1, K2], BF16, kind="ExternalInput").ap()
    mmd = nc.dram_tensor("mm", [K, HPC, K], BF16, kind="ExternalInput").ap()
    wmixTd = nc.dram_tensor("wmixT", [K2, C], BF16, kind="ExternalInput").ap()
    pbrd = pbcd = None
    if with_pbias:
        pbrd = nc.dram_tensor("pbr", [1, K2], BF16, kind="ExternalInput").ap()
        pbcd = nc.dram_tensor("pbc", [K2, 1], FP32, kind="ExternalInput").ap()
    outd = nc.dram_tensor("out", [W, C], BF16, kind="ExternalOutput").ap()
    with tile.TileContext(nc) as tc:
        with ExitStack() as ctx:
            _body(ctx, tc, x1td, x2d, pjd, colsumd, mmd, wmixTd, outd,
                  pbrd, pbcd)
    nc.compile()
    _PROGRAM_CACHE[key] = nc
    return nc


def _host_prep(inputs):
    x1 = np.asarray(inputs["x1"], np.float32)
    x2 = np.asarray(inputs["x2"], np.float32)
    gamma = np.asarray(inputs["gamma"], np.float32)
    beta = np.asarray(inputs["beta"], np.float32)
    proj = np.asarray(inputs["proj_nck"], np.float32)
    halves = np.asarray(inputs["halves"], np.float32)
    diagonals = np.asarray(inputs["diagonals"], np.float32)
    wmix = np.asarray(inputs["W_mixer"], np.float32)

    iu0, iu1 = np.triu_indices(K, k=1)
    m = np.zeros((N, K, K), np.float32)
    m[:, iu0, iu1] = halves
    m = m + np.swapaxes(m, -1, -2)
    d = np.arange(K)
    m[:, d, d] = diagonals

    pgam = proj * gamma[None, :, None]
    with_pbias = bool(np.any(beta))
    pbias = np.einsum("c,nck->nk", beta, pgam) if with_pbias else None

    x1t = [np.ascontiguousarray(x1[b].T.astype(BF)) for b in range(B)]
    x2b = [np.ascontiguousarray(x2[b].astype(BF)) for b in range(B)]

    in_maps = []
    for core in range(NCORES):
        b, hg = divmod(core, NCORES // B)
        h0 = HPC * hg
        pj_core = np.concatenate([pgam[h0 + i] for i in range(HPC)], axis=1)
        im = {
            "x1t": x1t[b],
            "x2": x2b[b],
            "pj": np.ascontiguousarray(pj_core.astype(BF)),
            "colsum": np.ascontiguousarray(
                pj_core.sum(axis=0)[None, :].astype(BF)),
            "mm": np.ascontiguousarray(
                np.stack([m[h0 + i] for i in range(HPC)], axis=1).astype(BF)),
            "wmixT": np.ascontiguousarray(
                wmix[:, K2 * hg:K2 * (hg + 1)].T.astype(BF)),
        }
        if with_pbias:
            pb = np.concatenate([pbias[h0 + i] for i in range(HPC)])
            im["pbr"] = np.ascontiguousarray(pb[None, :].astype(BF))
            im["pbc"] = np.ascontiguousarray(pb[:, None].astype(np.float32))
        in_maps.append(im)
    return in_maps, with_pbias


_TRACE = False
LAST_RESULT = None


def kernel(**inputs) -> np.ndarray:
    global LAST_RESULT
    in_maps, with_pbias = _host_prep(inputs)
    nc = _get_program(with_pbias)
    res = run_bass_kernel_spmd(nc, in_maps, core_ids=list(range(NCORES)),
                               trace=_TRACE)
    LAST_RESULT = res
    out = np.zeros((B, W, C), np.float32)
    for core in range(NCORES):
        b = core // (NCORES // B)
        out[b] += res.results[core]["out"].astype(np.float32)
    out += np.asarray(inputs["b_mixer"], np.float32)[None, None, :]
    return out
